# revision 1
# baseline (speedup 1.0000x reference)
"""Trainium2 Bass kernel for nn_Decoder (LSTM decoder + big output projection).

Model (VOCAB=32000, EM=256, UNITS=512, B=64, T=16):
  em     = emb_table[inputs]                      # [B,T,EM]
  xz     = em @ Wx + b                            # [B,T,4U] (precomputed input gates)
  scan:    z = xz_t + h @ Wh ; i,f,g,o = sigmoid(z)
           c = f*c + i*g ; h = o*sigmoid(c)       # 16 sequential steps
  logits = concat_t(h_t) @ Wout + bout            # [B, 8192] @ [8192, 32000]
  out    = softmax(logits)

Distribution over 8 NeuronCores:
  - The scan is replicated on every core (it is tiny and serial; replicating it
    avoids an AllGather of the hidden states).
  - Wout / bout are column-sharded: core c owns vocab columns
    [c*4000, (c+1)*4000).  Each core returns exp(logits_c) plus its local
    row-sum; the softmax denominator (the only cross-core quantity) is summed
    and divided on the host, so the device program has no collectives at all.
  - Weights are shipped to the device as bf16 (the 0.5 GB Wout read is the
    roofline floor; bf16 halves it vs f32).  Accumulation is fp32 in PSUM.
  - bout rides as two extra bf16 rows (value + residual) appended to the Wout
    stream, folded into the t=0 projection PSUM group by a K=2 matmul against
    a ones vector — no broadcast-bias tensor, no logits-init DMA.
  - All small inputs are packed into two tensors (one bf16, one f32): through
    the axon tunnel every extra buffer handle costs ~50us of per-call dispatch.

On-chip layout is "transposed": hidden state and gates live as [unit, batch]
tiles ([128 partitions, 64 batch]) so the recurrent matmul uses Wh as the
stationary operand and no per-step transposes are needed.  The per-step
output-projection partials (stationary = h_t^T, moving = streamed Wout rows)
run while the next step's gate math is on the Vector/Scalar engines, and the
Wout DMA stream overlaps everything.

_build_program(repeat=R) wraps the whole body in a hardware loop; test.py uses
that to measure per-execution device time with the tunnel's ~1.3 ms per-call
dispatch overhead amortized over R back-to-back executions.
"""

import numpy as np
import ml_dtypes
from contextlib import ExitStack

import concourse.bacc as bacc
import concourse.mybir as mybir
import concourse.tile as tile
from concourse.bass_utils import run_bass_kernel_spmd

VOCAB, EM, UNITS, B, T = 32000, 256, 512, 64, 16
NCORES = 8
VS = VOCAB // NCORES          # 4000 vocab columns per core
GU = 4 * UNITS                # 2048 gate units
NJ = GU // 128                # 16 gate m-tiles
KH = UNITS // 128             # 4 k-tiles of the hidden state
KE = EM // 128                # 2 k-tiles of the embedding
NTOK = B * T                  # 1024 tokens
NCH = 8                       # output-projection n-chunks per core
CH = VS // NCH                # 500 columns per chunk (<=512 PSUM bank limit)

# packed bf16 const tensor column offsets: wx | emt | wh | h0 | ident
OFF_WX = 0                    # [128, KE*GU]   = 4096 cols
OFF_EMT = OFF_WX + KE * GU    # [128, KE*NTOK] = 2048 cols
OFF_WH = OFF_EMT + KE * NTOK  # [128, KH*GU]   = 8192 cols
OFF_H0 = OFF_WH + KH * GU     # [128, KH*B]    =  256 cols
OFF_ID = OFF_H0 + KH * B      # [128, 128]     =  128 cols
CBF = OFF_ID + 128            # 14720 cols bf16 per partition (28.75 KiB)
NPAIR = T * UNITS // 256      # 32 two-k-tile blocks in the Wout stream
# packed f32 const tensor: bt | c0
OFF_BT = 0                    # [128, NJ]      =   16 cols
OFF_C0 = OFF_BT + NJ          # [128, KH*B]    =  256 cols
CF = OFF_C0 + KH * B          # 272 cols f32 per partition

BF16 = mybir.dt.bfloat16
F32 = mybir.dt.float32

_prog_cache = {}


def _build_program(repeat=1, _compile=True):
    """Trace + compile the single-core SPMD program (cached per process).

    repeat>1 wraps the whole body in a hardware loop (For_i) so one NEFF
    dispatch executes the kernel `repeat` times back to back — used only for
    timing; the outputs are simply overwritten each iteration."""
    key = (repeat, _compile)
    if key in _prog_cache:
        return _prog_cache[key]

    nc = bacc.Bacc("TRN2", target_bir_lowering=False, debug=False,
                   num_devices=NCORES)

    cbf_d = nc.dram_tensor("cbf", [128, CBF], BF16, kind="ExternalInput").ap()
    cf_d = nc.dram_tensor("cf", [128, CF], F32, kind="ExternalInput").ap()
    # Wout repacked host-side into pair-contiguous blocks: row k*128+p holds
    # [wout_row(2k*128+p) | wout_row((2k+1)*128+p)] so each streamed tile is
    # one fully contiguous 2MB DMA (16KiB per partition line).  The final row
    # 4096 is [bf16(bout) | bout - bf16(bout)], summed on-device via a K=2
    # ones matmul in the t=0 projection PSUM group.
    wout_d = nc.dram_tensor("wout", [NPAIR * 128 + 1, 2 * VS], BF16,
                            kind="ExternalInput").ap()
    # eps ships bf16: the softmax numerator tolerates 0.4% quantization (the
    # denominator is accumulated in f32 on-device before rounding), and it
    # halves the 1MB output writeback.
    eps_d = nc.dram_tensor("eps", [B, VS], BF16, kind="ExternalOutput").ap()
    ssum_d = nc.dram_tensor("ssum", [B, 1], F32, kind="ExternalOutput").ap()

    wout_r = wout_d[0:NPAIR * 128, :].rearrange("(k p) n -> k p n", p=128)
    bias_r = wout_d[NPAIR * 128:NPAIR * 128 + 1, :].rearrange(
        "q (s n) -> (q s) n", s=2)

    with tile.TileContext(nc) as tc, ExitStack() as ctx:
        consts = ctx.enter_context(tc.tile_pool(name="consts", bufs=1))
        wout_pool = ctx.enter_context(tc.tile_pool(name="wout", bufs=6))
        psum_big = ctx.enter_context(tc.tile_pool(name="psb", bufs=2, space="PSUM"))
        psum_proj = ctx.enter_context(tc.tile_pool(name="psp", bufs=4, space="PSUM"))
        work = ctx.enter_context(tc.tile_pool(name="work", bufs=1))

        def emit_body():
            # ---- resident tensors --------------------------------------------
            # cbf is split so the xz-phase inputs (wx+emt, the first 6144 cols)
            # land ~6us before the rest — the PE starts that much earlier, which
            # propagates to proj(0) releasing Wout-stream buffers in time.
            cbf = consts.tile([128, CBF], BF16, tag="cbf")
            nc.sync.dma_start(out=cbf[:, OFF_WX:OFF_WH], in_=cbf_d[:, OFF_WX:OFF_WH])
            cf = consts.tile([128, CF], F32, tag="cf")
            nc.sync.dma_start(out=cf[:], in_=cf_d[:])
            nc.sync.dma_start(out=cbf[:, OFF_WH:CBF], in_=cbf_d[:, OFF_WH:CBF])
            bias_sb = consts.tile([2, VS], BF16, tag="bias")
            nc.sync.dma_start(out=bias_sb[:], in_=bias_r[:])

            wx_sb = cbf[:, OFF_WX:OFF_EMT].rearrange("p (k g) -> p k g", k=KE)
            emt = cbf[:, OFF_EMT:OFF_WH].rearrange("p (k n) -> p k n", k=KE)
            wh_sb = cbf[:, OFF_WH:OFF_H0].rearrange("p (k g) -> p k g", k=KH)
            h0_sb = cbf[:, OFF_H0:OFF_ID].rearrange("p (k b) -> p k b", k=KH)
            id_sb = cbf[:, OFF_ID:OFF_ID + 128]
            bt_sb = cf[:, OFF_BT:OFF_C0]

            # cell state (f32, mutated in place every step)
            c_sb = consts.tile([128, KH * B], F32, tag="c")
            nc.vector.tensor_copy(c_sb[:], cf[:, OFF_C0:OFF_C0 + KH * B])
            # ones vector for the K=2 bias matmul
            ones2 = consts.tile([2, B], BF16, tag="ones2")
            nc.vector.memset(ones2[:], 1.0)
            # hidden states for all steps (slot 0 = initial state), bf16 transposed
            hs_sb = consts.tile([128, T + 1, KH, B], BF16, tag="hs")
            nc.vector.tensor_copy(
                hs_sb[:, 0, :, :].rearrange("p k b -> p (k b)"),
                h0_sb.rearrange("p k b -> p (k b)"))
            # logits accumulator (t=0 projection writes it, t>0 accumulate)
            logits = consts.tile([B, VS], F32, tag="logits")
            # xz = em @ Wx + b, transposed layout [gate-unit, (t, b)] stored
            # t-major so each scan step's xz slice is one contiguous [128, NJ*B]
            # block (enables the wide identity-matmul injection below)
            xz_sb = consts.tile([128, T, NJ, B], BF16, tag="xz")

            # ---- xz = Wx^T @ em^T  (+ b folded in during PSUM evacuation) ----
            for j in range(NJ):
                ps = psum_big.tile([128, T * B], F32, tag="zps")
                for kt in range(KE):
                    for nh in range(2):
                        nc.tensor.matmul(
                            ps[:, nh * 512:(nh + 1) * 512],
                            wx_sb[:, kt, j * 128:(j + 1) * 128],
                            emt[:, kt, nh * 512:(nh + 1) * 512],
                            start=(kt == 0), stop=(kt == KE - 1),
                        )
                nc.vector.tensor_scalar_add(
                    xz_sb[:, :, j, :],
                    ps.rearrange("p (t b) -> p t b", b=B),
                    bt_sb[:, j:j + 1])

            # ---- the scan + interleaved output projection --------------------
            # Emission order matters for the scheduler: within step t we emit
            # z(t) matmuls FIRST, then the projection for step t-1, then the
            # gate math for t.  That way the PE chews on proj(t-1) while the
            # Vector/Scalar engines run gates(t) — without this the PE idles
            # ~5us per step waiting for h(t).
            def emit_proj(t, wts):
                # logits (+)= h_t @ Wout[512t:512(t+1), :]; t==0 also folds in
                # bout (two bf16 rows against ones) and writes instead of adds.
                for j in range(NCH):
                    pp = psum_proj.tile([B, CH], F32, tag="pp")
                    if t == 0:
                        nc.tensor.matmul(
                            pp[:], ones2[:, :], bias_sb[:, j * CH:(j + 1) * CH],
                            start=True, stop=False)
                    for kt in range(KH):
                        nc.tensor.matmul(
                            pp[:],
                            hs_sb[:, t + 1, kt, :],
                            wts[kt // 2][:, kt % 2, j * CH:(j + 1) * CH],
                            start=(t != 0 and kt == 0), stop=(kt == KH - 1),
                        )
                    if t == 0:
                        nc.vector.tensor_copy(logits[:, j * CH:(j + 1) * CH], pp[:])
                    else:
                        nc.vector.tensor_tensor(
                            logits[:, j * CH:(j + 1) * CH], pp[:],
                            logits[:, j * CH:(j + 1) * CH], mybir.AluOpType.add)

            prev_wts = None
            for t in range(T):
                # stream this step's 512 Wout rows (2 halves of 2 k-tiles each).
                # The last step's tiles arrive as interleaved column slices so
                # proj(15) chunk j can start as soon as its columns land instead
                # of waiting for the whole 2MB tile — shaves ~10us off the tail.
                wt0 = wout_pool.tile([128, 2, VS], BF16, tag="wt")
                wt1 = wout_pool.tile([128, 2, VS], BF16, tag="wt")
                wts = [wt0, wt1]
                if t < T - 1:
                    for half in range(2):
                        nc.sync.dma_start(
                            out=wts[half][:].rearrange("p s n -> p (s n)"),
                            in_=wout_r[2 * t + half])
                else:
                    # eight 256KB column chunks per half, ordered so proj chunk j
                    # has both its k-tile halves as early as possible
                    for r in (0, 4, 1, 5, 2, 6, 3, 7):
                        s, c0 = r // 4, (r % 4) * 1000
                        for half in range(2):
                            nc.sync.dma_start(
                                out=wts[half][:, s, c0:c0 + 1000],
                                in_=wout_r[2 * t + half][:, r * 1000:(r + 1) * 1000])

                # At t==1 the Wout pool is near full and the scan still has ~60us
                # of slack, so release proj(0)'s buffers before starting z(1);
                # for later steps proj goes after z so the PE chews on it while
                # the Vector/Scalar engines run the gate math.
                if t == 1 and prev_wts is not None:
                    emit_proj(0, prev_wts)

                # z^T = xz_t + Wh^T @ h^T   [2048 units, 64 batch] in PSUM.
                # xz_t enters the accumulation group via an identity matmul so
                # no separate Vector-engine add sits on the serial chain.
                zt = psum_big.tile([128, NJ * B], F32, tag="zps")
                ztv = zt.rearrange("p (j b) -> p j b", b=B)
                # xz_t enters each bank with ONE wide 512-col identity matmul
                # (vs 16 per-j 64-col ones, or a cross-engine PSUM preload which
                # serializes each step behind a copy): the scan's per-step PE
                # time must stay under the 11.4us Wout-pair DMA cadence or the
                # PE queue backlog surfaces as a ~14us post-stream tail.
                for bank in range(2):
                    nc.tensor.matmul(
                        zt[:, bank * 512:(bank + 1) * 512], id_sb[:],
                        xz_sb[:, t, bank * 8:(bank + 1) * 8, :].rearrange(
                            "p j b -> p (j b)"),
                        start=True, stop=False)
                for j in range(NJ):
                    for kt in range(KH):
                        nc.tensor.matmul(
                            ztv[:, j, :],
                            wh_sb[:, kt, j * 128:(j + 1) * 128],
                            hs_sb[:, t, kt, :],
                            start=False, stop=(kt == KH - 1),
                        )
                # all four gates use sigmoid; host permuted gate columns to
                # [i, g, f, o] so the activation can run in two halves and the
                # i*g product starts while f/o are still on the Scalar engine
                a_sb = work.tile([128, NJ, B], F32, tag="a")
                nc.scalar.activation(a_sb[:, 0:8, :], ztv[:, 0:8, :],
                                     mybir.ActivationFunctionType.Sigmoid)
                nc.scalar.activation(a_sb[:, 8:16, :], ztv[:, 8:16, :],
                                     mybir.ActivationFunctionType.Sigmoid)
                iT = a_sb[:, 0:4, :].rearrange("p j b -> p (j b)")
                gT = a_sb[:, 4:8, :].rearrange("p j b -> p (j b)")
                fT = a_sb[:, 8:12, :].rearrange("p j b -> p (j b)")
                oT = a_sb[:, 12:16, :].rearrange("p j b -> p (j b)")
                t1 = work.tile([128, KH * B], F32, tag="t1")
                t2 = work.tile([128, KH * B], F32, tag="t2")
                nc.vector.tensor_mul(t1[:], iT, gT)
                nc.vector.tensor_mul(t2[:], fT, c_sb[:])
                nc.vector.tensor_add(c_sb[:], t1[:], t2[:])
                sc = work.tile([128, KH * B], F32, tag="sc")
                nc.scalar.activation(sc[:], c_sb[:],
                                     mybir.ActivationFunctionType.Sigmoid)
                nc.vector.tensor_mul(
                    hs_sb[:, t + 1, :, :].rearrange("p k b -> p (k b)"), oT, sc[:])
                if prev_wts is not None and t != 1:
                    emit_proj(t - 1, prev_wts)
                prev_wts = wts

            # ---- final projection step fused with the softmax numerator:
            # chunk j's exp (with a per-chunk row-sum accumulator) starts as soon
            # as its last evacuation lands, and its output DMA streams behind the
            # activation.  The denominator never crosses cores on device — the
            # host sums the 8 per-core ssum values and divides.
            eps_sb = wout_pool.tile([B, VS], BF16, tag="wt")
            ssum8 = consts.tile([B, NCH], F32, tag="ssum8")
            t = T - 1
            for j in range(NCH):
                pp = psum_proj.tile([B, CH], F32, tag="pp")
                for kt in range(KH):
                    nc.tensor.matmul(
                        pp[:],
                        hs_sb[:, t + 1, kt, :],
                        prev_wts[kt // 2][:, kt % 2, j * CH:(j + 1) * CH],
                        start=(kt == 0), stop=(kt == KH - 1),
                    )
                nc.vector.tensor_tensor(
                    logits[:, j * CH:(j + 1) * CH], pp[:],
                    logits[:, j * CH:(j + 1) * CH], mybir.AluOpType.add)
                nc.scalar.activation(
                    eps_sb[:, j * CH:(j + 1) * CH], logits[:, j * CH:(j + 1) * CH],
                    mybir.ActivationFunctionType.Exp, accum_out=ssum8[:, j:j + 1])
                nc.sync.dma_start(out=eps_d[:, j * CH:(j + 1) * CH],
                                  in_=eps_sb[:, j * CH:(j + 1) * CH])
            ssum = consts.tile([B, 1], F32, tag="ssum")
            nc.vector.reduce_sum(ssum[:], ssum8[:], axis=mybir.AxisListType.X)
            nc.sync.dma_start(out=ssum_d[:], in_=ssum[:])

        # repeat>1: two kernel executions per hardware-loop iteration
        # halves the all-engine barrier/reset cost, and lets body 2's
        # const+Wout DMA stream start while body 1's tail drains (the
        # tile pools' WAR tracking orders the overlap correctly).
        if repeat > 1:
            assert repeat % 2 == 0
            with tc.For_i(0, repeat // 2):
                emit_body()
                emit_body()
        else:
            emit_body()

    if _compile:
        nc.compile()
    _prog_cache[key] = nc
    return nc


def _prep_in_maps(inputs):
    bf = ml_dtypes.bfloat16
    tok = np.asarray(inputs["inputs"]).astype(np.int64)        # [B, T]
    enc_h = np.asarray(inputs["enc_h"], np.float32)            # [B, U]
    enc_c = np.asarray(inputs["enc_c"], np.float32)            # [B, U]
    emb = np.asarray(inputs["emb_table"], np.float32)          # [V, EM]
    Wx = np.asarray(inputs["Wx"], np.float32)                  # [EM, 4U]
    Wh = np.asarray(inputs["Wh"], np.float32)                  # [U, 4U]
    b = np.asarray(inputs["b"], np.float32)                    # [4U]
    Wout = np.asarray(inputs["Wout"], np.float32)              # [T*U, V]
    bout = np.asarray(inputs["bout"], np.float32)              # [V]

    # embedding lookup on host (pure data movement), shipped pre-transposed:
    # emt[p, k, i] = emb[tok_i, k*128 + p] with token order i = t*B + b
    em_flat = emb[tok.T.reshape(-1)]                           # [NTOK, EM]
    emt = em_flat.reshape(NTOK, KE, 128).transpose(2, 1, 0).astype(bf)

    # permute gate columns i,f,g,o -> i,g,f,o (lets the device split the
    # sigmoid into [i,g] / [f,o] halves)
    perm = np.r_[0:UNITS, 2 * UNITS:3 * UNITS, UNITS:2 * UNITS, 3 * UNITS:GU]
    Wx = Wx[:, perm]
    Wh = Wh[:, perm]
    b = b[perm]

    cbf = np.empty((128, CBF), dtype=bf)
    cbf[:, OFF_WX:OFF_EMT] = (
        Wx.reshape(KE, 128, GU).transpose(1, 0, 2).reshape(128, KE * GU))
    cbf[:, OFF_EMT:OFF_WH] = emt.reshape(128, KE * NTOK)
    cbf[:, OFF_WH:OFF_H0] = (
        Wh.reshape(KH, 128, GU).transpose(1, 0, 2).reshape(128, KH * GU))
    cbf[:, OFF_H0:OFF_ID] = (
        enc_h.T.reshape(KH, 128, B).transpose(1, 0, 2).reshape(128, KH * B))
    cbf[:, OFF_ID:OFF_ID + 128] = np.eye(128, dtype=bf)

    cfl = np.empty((128, CF), dtype=np.float32)
    cfl[:, OFF_BT:OFF_C0] = b.reshape(NJ, 128).T
    cfl[:, OFF_C0:OFF_C0 + KH * B] = (
        enc_c.T.reshape(KH, 128, B).transpose(1, 0, 2).reshape(128, KH * B))

    common = {"cbf": np.ascontiguousarray(cbf), "cf": cfl}
    wout_bf = Wout.astype(bf)
    bout_hi = bout.astype(bf)
    bout_lo = (bout - bout_hi.astype(np.float32)).astype(bf)
    in_maps = []
    for c in range(NCORES):
        m = dict(common)
        # pair-contiguous repack: block k row p = [row(2k*128+p) | row((2k+1)*128+p)]
        wv = wout_bf[:, c * VS:(c + 1) * VS].reshape(NPAIR, 2, 128, VS)
        w = np.empty((NPAIR * 128 + 1, 2 * VS), dtype=bf)
        w[:NPAIR * 128] = wv.transpose(0, 2, 1, 3).reshape(NPAIR * 128, 2 * VS)
        w[NPAIR * 128, :VS] = bout_hi[c * VS:(c + 1) * VS]
        w[NPAIR * 128, VS:] = bout_lo[c * VS:(c + 1) * VS]
        m["wout"] = w
        in_maps.append(m)
    return in_maps


def _run(inputs, trace=False):
    nc = _build_program()
    in_maps = _prep_in_maps(inputs)
    res = run_bass_kernel_spmd(nc, in_maps, list(range(NCORES)), trace=trace)
    eps = np.concatenate(
        [res.results[c]["eps"].astype(np.float32) for c in range(NCORES)], axis=1)
    denom = np.sum([res.results[c]["ssum"] for c in range(NCORES)], axis=0)
    out = eps / denom
    return out.astype(np.float32), res


def kernel(**inputs) -> np.ndarray:
    out, _ = _run(inputs, trace=False)
    return out



# revision 14
# speedup vs baseline: 1.4204x; 1.4204x over previous
"""Trainium2 Bass kernel for nn_Decoder (LSTM decoder + big output projection).

Model (VOCAB=32000, EM=256, UNITS=512, B=64, T=16):
  em     = emb_table[inputs]                      # [B,T,EM]
  scan:    z = em_t @ Wx + b + h @ Wh ; i,f,g,o = sigmoid(z)
           c = f*c + i*g ; h = o*sigmoid(c)       # 16 sequential steps
  logits = concat_t(h_t) @ Wout + bout            # [B, 8192] @ [8192, 32000]
  out    = softmax(logits)

Distribution over 8 NeuronCores: the scan is replicated on every core (tiny,
serial); Wout / bout are column-sharded (core c owns vocab columns
[c*4000, (c+1)*4000)).  Each core returns exp(logits_c) plus its local
row-sums; the softmax denominator is summed and divided on the host, so the
device program has no collectives at all.

Key techniques (v2):
  - Wout ships as fp8e4 (x1024 scale), streamed and fed straight into the
    projection matmuls as the *moving* operand against bf16 hidden states
    (mixed-dtype matmul).  This halves the dominant HBM stream (65.5 -> 32.8
    MB/core).  The fp8 quantization error is killed by a rank-1 correction:
    logits += (flat . u) * (u^T (Wout - W8)) for a fixed unit vector u chosen
    along the mean hidden-state direction (the LSTM's sigmoid gates
    concentrate h tightly around its mean, so u captures ~99% of flat's
    energy; the correction cuts the rel-err from 2.7e-2 to ~3e-3).  u is
    derived from a surrogate scan on random *fake* inputs (same
    distribution), so it is input-independent; v = u^T (Wout - W8) rides as
    one extra bf16 row next to the two bias rows.
  - Logits live in PSUM for the whole kernel: chunk 2b accumulates in bank
    b partitions 0:64, chunk 2b+1 in partitions 64:128 (tile_position picks
    the PE column-group from the out AP's base partition, so the pair of
    128x64-stationary matmuls runs concurrently in disjoint halves of the
    PE array).  No per-step PSUM evacuation / DVE accumulation at all; the
    final exp reads PSUM directly, with scale=1/1024 folding the fp8
    descale, and bias+v enter as a single K=3 matmul per chunk at the end
    (stationary rows = [1, 1, s_b]).
  - The xz precompute phase is fused into the scan: each step's z-group
    accumulates Wx^T em_t alongside Wh^T h in PSUM, so there is no xz
    SBUF buffer, no identity-injection matmuls and no evacuation pass.
  - s = flat . u accumulates in a [1,64] PSUM via 4 tiny matmuls per step
    riding behind the projection.
  - All small inputs are packed into few tensors; through the axon tunnel
    every extra buffer handle costs ~50us of per-call dispatch.

_build_program(repeat=R) wraps the whole body in a hardware loop; test.py
uses that to measure per-execution device time with the tunnel's ~1.3 ms
per-call dispatch overhead amortized over R back-to-back executions.
"""

import numpy as np
import ml_dtypes
from contextlib import ExitStack

import concourse.bacc as bacc
import concourse.mybir as mybir
import concourse.tile as tile
from concourse.bass_utils import run_bass_kernel_spmd

VOCAB, EM, UNITS, B, T = 32000, 256, 512, 64, 16
NCORES = 8
VS = VOCAB // NCORES          # 4000 vocab columns per core
GU = 4 * UNITS                # 2048 gate units
NJ = GU // 128                # 16 gate m-tiles
KH = UNITS // 128             # 4 k-tiles of the hidden state
KE = EM // 128                # 2 k-tiles of the embedding
NTOK = B * T                  # 1024 tokens
NCH = 8                       # output-projection chunks per core
CH = VS // NCH                # 500 columns per chunk (<=512 PSUM bank limit)
NBK = NCH // 2                # 4 PSUM banks hold all 8 chunks (2 per bank)
NPAIR = T * UNITS // 256      # 32 two-k-tile blocks in the Wout stream
WSCALE = 1024.0               # fp8 shipping scale for Wout (+bias/v rows)

# packed bf16 const tensor column offsets: wx | emt | wh | h0 | u
OFF_WX = 0                    # [128, KE*GU]   = 4096 cols
OFF_EMT = OFF_WX + KE * GU    # [128, KE*NTOK] = 2048 cols
OFF_WH = OFF_EMT + KE * NTOK  # [128, KH*GU]   = 8192 cols
OFF_H0 = OFF_WH + KH * GU     # [128, KH*B]    =  256 cols
OFF_U = OFF_H0 + KH * B       # [128, T*KH]    =   64 cols
CBF = OFF_U + T * KH          # 14656 cols bf16 per partition (28.6 KiB)
# packed f32 const tensor: bt | c0
OFF_BT = 0                    # [128, NJ]      =   16 cols
OFF_C0 = OFF_BT + NJ          # [128, KH*B]    =  256 cols
CF = OFF_C0 + KH * B          # 272 cols f32 per partition

BF16 = mybir.dt.bfloat16
F32 = mybir.dt.float32
FP8 = mybir.dt.float8e4

_prog_cache = {}


def _build_program(repeat=1, _compile=True, zero_b=True):
    """Trace + compile the single-core SPMD program (cached per process).

    repeat>1 wraps the whole body in a hardware loop (For_i) so one NEFF
    dispatch executes the kernel `repeat` times back to back — used only for
    timing; the outputs are simply overwritten each iteration.
    zero_b: the gate bias b is all-zero (true for this model), letting the
    per-step sigmoids run as two wide halves instead of 16 per-j calls."""
    key = (repeat, _compile, zero_b)
    if key in _prog_cache:
        return _prog_cache[key]

    nc = bacc.Bacc("TRN2", target_bir_lowering=False, debug=False,
                   num_devices=NCORES)

    cbf_d = nc.dram_tensor("cbf", [128, CBF], BF16, kind="ExternalInput").ap()
    cf_d = nc.dram_tensor("cf", [128, CF], F32, kind="ExternalInput").ap()
    # Wout repacked host-side into pair-contiguous fp8 blocks: row k*128+p
    # holds [wout_row(2k*128+p) | wout_row((2k+1)*128+p)] * WSCALE in fp8e4,
    # so each streamed tile is one fully contiguous 1MB DMA.
    wout_d = nc.dram_tensor("wout", [NPAIR * 128, 2 * VS], FP8,
                            kind="ExternalInput").ap()
    # bias/correction rows (scaled domain): [bout_hi; bout_lo; v] * WSCALE
    bv_d = nc.dram_tensor("bv", [3, VS], BF16, kind="ExternalInput").ap()
    # eps ships bf16: the softmax numerator tolerates 0.4% quantization (the
    # denominator is accumulated in f32 on-device before rounding).
    eps_d = nc.dram_tensor("eps", [B, VS], BF16, kind="ExternalOutput").ap()
    # per-partition row sums: partitions 0:64 = even chunks, 64:128 = odd;
    # the host adds the two halves (and across cores) for the denominator.
    ssum_d = nc.dram_tensor("ssum", [128, 1], F32, kind="ExternalOutput").ap()

    wout_r = wout_d.rearrange("(k p) n -> k p n", p=128)

    with tile.TileContext(nc) as tc, ExitStack() as ctx:
        consts = ctx.enter_context(tc.tile_pool(name="consts", bufs=2))
        wout_pool = ctx.enter_context(tc.tile_pool(name="wout", bufs=8))
        psum_proj = ctx.enter_context(tc.tile_pool(name="psp", bufs=1, space="PSUM"))
        psum_z = ctx.enter_context(tc.tile_pool(name="psz", bufs=2, space="PSUM"))
        psum_s = ctx.enter_context(tc.tile_pool(name="pss", bufs=1, space="PSUM"))
        work = ctx.enter_context(tc.tile_pool(name="work", bufs=1))

        def emit_body():
            # ---- resident tensors --------------------------------------------
            # wx+emt land first so step-0's z matmuls start ASAP; the rest of
            # cbf (wh onward) follows on the same queue.
            cbf = consts.tile([128, CBF], BF16, tag="cbf")
            nc.sync.dma_start(out=cbf[:, OFF_WX:OFF_WH], in_=cbf_d[:, OFF_WX:OFF_WH])
            cf = consts.tile([128, CF], F32, tag="cf")
            nc.sync.dma_start(out=cf[:], in_=cf_d[:])
            nc.sync.dma_start(out=cbf[:, OFF_WH:CBF], in_=cbf_d[:, OFF_WH:CBF])
            bv_sb = consts.tile([3, VS], BF16, tag="bv")
            nc.sync.dma_start(out=bv_sb[:], in_=bv_d[:])

            wx_sb = cbf[:, OFF_WX:OFF_EMT].rearrange("p (k g) -> p k g", k=KE)
            emt = cbf[:, OFF_EMT:OFF_WH].rearrange("p (k n) -> p k n", k=KE)
            wh_sb = cbf[:, OFF_WH:OFF_H0].rearrange("p (k g) -> p k g", k=KH)
            h0_sb = cbf[:, OFF_H0:OFF_U].rearrange("p (k b) -> p k b", k=KH)
            u_sb = cbf[:, OFF_U:OFF_U + T * KH]
            bt_sb = cf[:, OFF_BT:OFF_C0]

            # cell state (f32, mutated in place every step)
            c_sb = consts.tile([128, KH * B], F32, tag="c")
            nc.vector.tensor_copy(c_sb[:], cf[:, OFF_C0:OFF_C0 + KH * B])
            # stationary for the final bias+v matmul: rows [s_b, 1, 1]
            # (s in row 0 keeps the PSUM->SBUF copy partition-aligned)
            stat3 = consts.tile([3, B], BF16, tag="stat3")
            nc.vector.memset(stat3[:], 1.0)
            # hidden states for all steps (slot 0 = initial state), bf16
            hs_sb = consts.tile([128, T + 1, KH, B], BF16, tag="hs")
            nc.vector.tensor_copy(
                hs_sb[:, 0, :, :].rearrange("p k b -> p (k b)"),
                h0_sb.rearrange("p k b -> p (k b)"))

            # logits accumulators: bank b holds chunk 2b in partitions 0:64
            # and chunk 2b+1 in partitions 64:128, resident in PSUM for the
            # whole kernel.
            pp = [psum_proj.tile([128, 512], F32, tag=f"pp{b}", name=f"pp{b}")
                  for b in range(NBK)]
            # s = flat . u accumulator
            s_ps = psum_s.tile([1, B], F32, tag="s")

            def emit_proj(t, wts):
                # logits (+)= h_t @ Wout[512t:512(t+1), :].  Chunk pair
                # (2b, 2b+1) shares the hs_kt stationary; the odd chunk's out
                # lands in psum partitions 64:128 (PE column-group 2-3), so
                # the two matmuls run concurrently in disjoint array halves.
                for kt in range(KH):
                    h_st = hs_sb[:, t + 1, kt, :]
                    w_mv = wts[kt // 2][:, kt % 2, :]
                    for b in range(NBK):
                        nc.tensor.matmul(
                            pp[b][0:B, 0:CH], h_st,
                            w_mv[:, (2 * b) * CH:(2 * b + 1) * CH],
                            start=(t == 0 and kt == 0), stop=False,
                            skip_group_check=True)
                        nc.tensor.matmul(
                            pp[b][B:2 * B, 0:CH], h_st,
                            w_mv[:, (2 * b + 1) * CH:(2 * b + 2) * CH],
                            start=(t == 0 and kt == 0), stop=False,
                            skip_group_check=True)

            def emit_s(slot, start, stop):
                # s_b += u[tile] . h[tile] for the 4 k-tiles of scan slot
                for kt in range(KH):
                    nc.tensor.matmul(
                        s_ps[:], u_sb[:, (slot - 1) * KH + kt:(slot - 1) * KH + kt + 1],
                        hs_sb[:, slot, kt, :],
                        start=(start and kt == 0), stop=(stop and kt == KH - 1),
                        skip_group_check=True)

            prev_wts = None
            for t in range(T):
                # stream this step's 512 Wout rows (2 k-tile pairs)
                wt0 = wout_pool.tile([128, 2, VS], FP8, tag="wt")
                wt1 = wout_pool.tile([128, 2, VS], FP8, tag="wt")
                wts = [wt0, wt1]
                if t < T - 1:
                    for half in range(2):
                        nc.sync.dma_start(
                            out=wts[half][:].rearrange("p s n -> p (s n)"),
                            in_=wout_r[2 * t + half])
                else:
                    # final step: stream in bank-pair column order so the
                    # last projection's chunks start as soon as their
                    # columns land
                    for r in range(NBK):
                        for half in range(2):
                            for s in range(2):
                                nc.sync.dma_start(
                                    out=wts[half][:, s, r * 1000:(r + 1) * 1000],
                                    in_=wout_r[2 * t + half][
                                        :, s * VS + r * 1000:s * VS + (r + 1) * 1000])

                # z^T = Wx^T em_t + b + Wh^T h   [2048 units, 64 batch],
                # two PSUM-bank halves of 8 gate tiles each
                zhs = []
                for half in range(2):
                    zh = psum_z.tile([128, 8, B], F32, tag="zh")
                    zhs.append(zh)
                    for j8 in range(8):
                        j = half * 8 + j8
                        for ke in range(KE):
                            nc.tensor.matmul(
                                zh[:, j8, :],
                                wx_sb[:, ke, j * 128:(j + 1) * 128],
                                emt[:, ke, t * B:(t + 1) * B],
                                start=(ke == 0), stop=False)
                        for kt in range(KH):
                            nc.tensor.matmul(
                                zh[:, j8, :],
                                wh_sb[:, kt, j * 128:(j + 1) * 128],
                                hs_sb[:, t, kt, :],
                                start=False, stop=(kt == KH - 1))

                # projection for the previous step rides behind z(t) on the
                # PE while the Vector/Scalar engines run this step's gates
                if prev_wts is not None:
                    emit_proj(t - 1, prev_wts)
                    emit_s(t, start=(t == 1), stop=False)

                # all four gates use sigmoid; host permuted gate columns to
                # [i, g, f, o] so the activation can run in two halves and
                # the i*g product starts while f/o are still pending
                a_sb = work.tile([128, NJ, B], F32, tag="a")
                if zero_b:
                    for half in range(2):
                        nc.scalar.activation(
                            a_sb[:, half * 8:(half + 1) * 8, :], zhs[half][:],
                            mybir.ActivationFunctionType.Sigmoid)
                else:
                    for half in range(2):
                        for j8 in range(8):
                            j = half * 8 + j8
                            nc.scalar.activation(
                                a_sb[:, j, :], zhs[half][:, j8, :],
                                mybir.ActivationFunctionType.Sigmoid,
                                bias=bt_sb[:, j:j + 1])
                iT = a_sb[:, 0:4, :].rearrange("p j b -> p (j b)")
                gT = a_sb[:, 4:8, :].rearrange("p j b -> p (j b)")
                fT = a_sb[:, 8:12, :].rearrange("p j b -> p (j b)")
                oT = a_sb[:, 12:16, :].rearrange("p j b -> p (j b)")
                t1 = work.tile([128, KH * B], F32, tag="t1")
                t2 = work.tile([128, KH * B], F32, tag="t2")
                nc.vector.tensor_mul(t1[:], iT, gT)
                nc.vector.tensor_mul(t2[:], fT, c_sb[:])
                nc.vector.tensor_add(c_sb[:], t1[:], t2[:])
                sc = work.tile([128, KH * B], F32, tag="sc")
                nc.scalar.activation(sc[:], c_sb[:],
                                     mybir.ActivationFunctionType.Sigmoid)
                nc.vector.tensor_mul(
                    hs_sb[:, t + 1, :, :].rearrange("p k b -> p (k b)"), oT, sc[:])
                prev_wts = wts

            # ---- tail: last projection, bias+v, fused exp ---------------------
            emit_proj(T - 1, prev_wts)
            emit_s(T, start=False, stop=True)
            # s -> bf16 stationary row
            nc.vector.tensor_copy(stat3[0:1, :], s_ps[:])
            eps_sb = consts.tile([128, NBK * CH], BF16, tag="eps")
            ssum8 = consts.tile([128, NBK], F32, tag="ssum8")
            for b in range(NBK):
                for half in range(2):
                    ch = 2 * b + half
                    nc.tensor.matmul(
                        pp[b][half * B:(half + 1) * B, 0:CH], stat3[:],
                        bv_sb[:, ch * CH:(ch + 1) * CH],
                        start=False, stop=True, skip_group_check=True)
                nc.scalar.activation(
                    eps_sb[:, b * CH:(b + 1) * CH], pp[b][:, 0:CH],
                    mybir.ActivationFunctionType.Exp, scale=1.0 / WSCALE,
                    accum_out=ssum8[:, b:b + 1])
                for half in range(2):
                    ch = 2 * b + half
                    nc.sync.dma_start(
                        out=eps_d[:, ch * CH:(ch + 1) * CH],
                        in_=eps_sb[half * B:(half + 1) * B, b * CH:(b + 1) * CH])
            ssum = consts.tile([128, 1], F32, tag="ssum")
            nc.vector.reduce_sum(ssum[:], ssum8[:], axis=mybir.AxisListType.X)
            nc.sync.dma_start(out=ssum_d[:], in_=ssum[:])

        # repeat>1: two kernel executions per hardware-loop iteration
        # halves the all-engine barrier/reset cost, and lets body 2's
        # const+Wout DMA stream start while body 1's tail drains
        if repeat > 1:
            assert repeat % 2 == 0
            with tc.For_i(0, repeat // 2):
                emit_body()
                emit_body()
        else:
            emit_body()

    if _compile:
        nc.compile()
    _prog_cache[key] = nc
    return nc


def _surrogate_u(emb, Wx, Wh, b):
    """Mean hidden-state direction from a scan over random fake inputs
    (input-independent; uses only the weights)."""
    rng = np.random.default_rng(12345)
    tok = rng.integers(0, VOCAB, size=(B, T))
    h = rng.standard_normal((B, UNITS)).astype(np.float32)
    c = rng.standard_normal((B, UNITS)).astype(np.float32)
    em = emb[tok]                                       # [B,T,EM]
    xz = np.einsum("bte,eu->btu", em, Wx) + b
    hs = []
    for t in range(T):
        z = xz[:, t] + h @ Wh
        i, f, g, o = np.split(1.0 / (1.0 + np.exp(-z)), 4, axis=-1)
        c = f * c + i * g
        h = o / (1.0 + np.exp(-c))
        hs.append(h)
    u = np.stack(hs, 0).mean(1).reshape(-1)             # [T*UNITS]
    return (u / np.linalg.norm(u)).astype(np.float32)


def _prep_in_maps(inputs):
    bf = ml_dtypes.bfloat16
    f8 = ml_dtypes.float8_e4m3
    tok = np.asarray(inputs["inputs"]).astype(np.int64)        # [B, T]
    enc_h = np.asarray(inputs["enc_h"], np.float32)            # [B, U]
    enc_c = np.asarray(inputs["enc_c"], np.float32)            # [B, U]
    emb = np.asarray(inputs["emb_table"], np.float32)          # [V, EM]
    Wx = np.asarray(inputs["Wx"], np.float32)                  # [EM, 4U]
    Wh = np.asarray(inputs["Wh"], np.float32)                  # [U, 4U]
    b = np.asarray(inputs["b"], np.float32)                    # [4U]
    Wout = np.asarray(inputs["Wout"], np.float32)              # [T*U, V]
    bout = np.asarray(inputs["bout"], np.float32)              # [V]

    u = _surrogate_u(emb, Wx, Wh, b)                           # [T*U]

    # embedding lookup on host (pure data movement), shipped pre-transposed:
    # emt[p, k, i] = emb[tok_i, k*128 + p] with token order i = t*B + b
    em_flat = emb[tok.T.reshape(-1)]                           # [NTOK, EM]
    emt = em_flat.reshape(NTOK, KE, 128).transpose(2, 1, 0).astype(bf)

    # permute gate columns i,f,g,o -> i,g,f,o (lets the device split the
    # sigmoid into [i,g] / [f,o] halves)
    perm = np.r_[0:UNITS, 2 * UNITS:3 * UNITS, UNITS:2 * UNITS, 3 * UNITS:GU]
    Wx = Wx[:, perm]
    Wh = Wh[:, perm]
    bp = b[perm]

    cbf = np.empty((128, CBF), dtype=bf)
    cbf[:, OFF_WX:OFF_EMT] = (
        Wx.reshape(KE, 128, GU).transpose(1, 0, 2).reshape(128, KE * GU))
    cbf[:, OFF_EMT:OFF_WH] = emt.reshape(128, KE * NTOK)
    cbf[:, OFF_WH:OFF_H0] = (
        Wh.reshape(KH, 128, GU).transpose(1, 0, 2).reshape(128, KH * GU))
    cbf[:, OFF_H0:OFF_U] = (
        enc_h.T.reshape(KH, 128, B).transpose(1, 0, 2).reshape(128, KH * B))
    # u column (t*KH + kt), row p  <->  flat index t*512 + kt*128 + p
    cbf[:, OFF_U:OFF_U + T * KH] = (
        u.reshape(T * KH, 128).T.astype(bf))

    cfl = np.empty((128, CF), dtype=np.float32)
    cfl[:, OFF_BT:OFF_C0] = bp.reshape(NJ, 128).T
    cfl[:, OFF_C0:OFF_C0 + KH * B] = (
        enc_c.T.reshape(KH, 128, B).transpose(1, 0, 2).reshape(128, KH * B))

    common = {"cbf": np.ascontiguousarray(cbf), "cf": cfl}
    wout_f8 = np.clip(Wout * WSCALE, -240.0, 240.0).astype(f8)
    resid = Wout - wout_f8.astype(np.float32) / WSCALE          # [T*U, V]
    v = (u @ resid) * WSCALE                                    # [V]
    bout_s = bout * WSCALE
    bout_hi = bout_s.astype(bf)
    bout_lo = (bout_s - bout_hi.astype(np.float32)).astype(bf)
    in_maps = []
    for c in range(NCORES):
        m = dict(common)
        # pair-contiguous repack: block k row p = [row(2k*128+p) | row((2k+1)*128+p)]
        wv = wout_f8[:, c * VS:(c + 1) * VS].reshape(NPAIR, 2, 128, VS)
        m["wout"] = np.ascontiguousarray(
            wv.transpose(0, 2, 1, 3).reshape(NPAIR * 128, 2 * VS))
        bv = np.empty((3, VS), dtype=bf)
        bv[0] = v[c * VS:(c + 1) * VS].astype(bf)
        bv[1] = bout_hi[c * VS:(c + 1) * VS]
        bv[2] = bout_lo[c * VS:(c + 1) * VS]
        m["bv"] = bv
        in_maps.append(m)
    return in_maps


def _run(inputs, trace=False):
    zero_b = not np.any(np.asarray(inputs["b"], np.float32))
    nc = _build_program(zero_b=zero_b)
    in_maps = _prep_in_maps(inputs)
    res = run_bass_kernel_spmd(nc, in_maps, list(range(NCORES)), trace=trace)
    eps = np.concatenate(
        [res.results[c]["eps"].astype(np.float32) for c in range(NCORES)], axis=1)
    denom = np.sum([res.results[c]["ssum"][0:B, 0] + res.results[c]["ssum"][B:2 * B, 0]
                    for c in range(NCORES)], axis=0).reshape(B, 1)
    out = eps / denom
    return out.astype(np.float32), res


def kernel(**inputs) -> np.ndarray:
    out, _ = _run(inputs, trace=False)
    return out
